# revision 12
# baseline (speedup 1.0000x reference)
"""Adaptive-softmax logits (shortlist head + 2 tail clusters) on 8 TRN2 NeuronCores.

Sharding: head GEMM is data-parallel over tokens (1024 rows/core); both tail
clusters are vocab-sharded (each core computes 1/8 of each tail cluster's
columns for all tail tokens). No collectives: the host gathers tail rows,
packs per-core operands, and concatenates per-core outputs.

All device GEMMs run as out = lhsT.T @ rhs with bf16 operands and f32 PSUM
accumulation. Algebra: out0 = (h0 @ W0.T) @ (E0 @ W0.T).T = h0 @ (E0 @ P0).T
with P0 = W0.T @ W0 computed on host, so the per-token projection GEMM is not
replicated across cores. Biases are added on host during assembly; outputs
leave the device as bf16 to halve write bandwidth.

DMA engine split: weight/kxn streams and output stores ride the sync HWDGE
ring; resident and kxm loads ride the scalar HWDGE ring, so a stream with
stalled buffer slots can't head-of-line-block the other ring's loads.
"""

import sys
import types

import numpy as np
import ml_dtypes

BF16 = ml_dtypes.bfloat16

# Problem constants (hardcoded per task instructions).
N_CORES = 8
B, H, V = 8192, 1024, 50000
C0, C1, C2 = 2000, 20000, 50000   # cutoffs
V0 = C1 - C0                      # 18000 tail-0 vocab
V1 = C2 - C1                      # 30000 tail-1 vocab
V0_LOC = V0 // N_CORES            # 2250 per-core tail-0 columns
V1_LOC = V1 // N_CORES            # 3750
HEAD_N = 2002                     # 2000 shortlist + 2 tail logits
B_LOC = B // N_CORES              # 1024
KS = H // 128                     # 8 k-subtiles for K=1024
K1 = 256                          # contraction dim of tail-1 decode
KS1 = K1 // 128                   # 2

TRACE = False          # set True (e.g. from test.py) to neuron-profile the run
LAST_EXEC_NS = None    # filled when TRACE is on

_GRAPH_CACHE = {}


def _install_ntff_shim():
    """Register the NTFF profile hook bass_utils expects under axon.

    The agent image's ``antenv`` lacks ``axon_hooks``; recreate it in
    sys.modules backed by the ctypes driver from trn_agent_boot.
    """
    if "antenv.axon_hooks" in sys.modules:
        return
    m = types.ModuleType("antenv.axon_hooks")

    def set_axon_ntff_profile_hook(h):
        m._hook = h

    def get_axon_ntff_profile_hook():
        return getattr(m, "_hook", None)

    m.set_axon_ntff_profile_hook = set_axon_ntff_profile_hook
    m.get_axon_ntff_profile_hook = get_axon_ntff_profile_hook
    try:
        import antenv

        sys.modules["antenv.axon_hooks"] = m
        antenv.axon_hooks = m
        from trn_agent_boot.trn_boot import _ntff_profile_via_ctypes

        set_axon_ntff_profile_hook(
            _ntff_profile_via_ctypes("/opt/axon/libaxon_pjrt.so")
        )
    except Exception:
        pass


def _kx_tiled(a):
    """(K, M) -> (128, K//128, M) bf16: K on partitions, tiled by 128."""
    k, mdim = a.shape
    return np.ascontiguousarray(
        a.reshape(k // 128, 128, mdim).transpose(1, 0, 2)
    ).astype(BF16)


def _kx_mtiled(a):
    """(K, M) -> (M//128, 128, K//128, 128) bf16: one contiguous block per m-tile."""
    k, mdim = a.shape
    t = a.reshape(k // 128, 128, mdim // 128, 128)
    return np.ascontiguousarray(t.transpose(2, 1, 0, 3)).astype(BF16)


def _n_widths(total, tile):
    out = []
    off = 0
    while off < total:
        out.append(min(tile, total - off))
        off += tile
    return out


def _build_graph(n0p, n1p):
    import concourse.mybir as mybir
    import concourse.tile as tile
    from concourse import bacc
    from concourse.bass import ds, ts

    dt = mybir.dt
    m0 = n0p // 128
    m1 = n1p // 128

    nc = bacc.Bacc(None, target_bir_lowering=False, debug=False)

    hid_t = nc.declare_dram_parameter("hidT", [128, KS, B_LOC], dt.bfloat16, isOutput=False)
    wcat_t = nc.declare_dram_parameter("wcatT", [128, KS, HEAD_N], dt.bfloat16, isOutput=False)
    p0_t = nc.declare_dram_parameter("p0T", [128, KS, H], dt.bfloat16, isOutput=False)
    e0_t = nc.declare_dram_parameter("e0T", [128, KS, V0_LOC], dt.bfloat16, isOutput=False)
    h0_t = nc.declare_dram_parameter("h0T", [m0, 128, KS, 128], dt.bfloat16, isOutput=False)
    d1w_t = nc.declare_dram_parameter("d1WT", [128, KS, K1], dt.bfloat16, isOutput=False)
    e1_t = nc.declare_dram_parameter("e1T", [128, KS, V1_LOC], dt.bfloat16, isOutput=False)
    p1_t = nc.declare_dram_parameter("p1T", [m1, 128, KS1, 128], dt.bfloat16, isOutput=False)

    head_o = nc.declare_dram_parameter("head_o", [B_LOC, HEAD_N], dt.bfloat16, isOutput=True)
    out0_o = nc.declare_dram_parameter("out0_o", [n0p, V0_LOC], dt.bfloat16, isOutput=True)
    out1_o = nc.declare_dram_parameter("out1_o", [n1p, V1_LOC], dt.bfloat16, isOutput=True)

    wh = _n_widths(HEAD_N, 512)    # [512, 512, 512, 466]
    w0 = _n_widths(V0_LOC, 512)    # [512 x4, 202]
    w1 = _n_widths(V1_LOC, 512)    # [512 x7, 166]

    with (
        tile.TileContext(nc) as tc,
        tc.tile_pool(name="res", bufs=1) as res,
        tc.tile_pool(name="kxn", bufs=4) as kxn_pool,
        tc.tile_pool(name="kxm", bufs=6) as kxm_pool,
        tc.tile_pool(name="stage", bufs=3) as stage_pool,
        tc.tile_pool(name="psum", bufs=8, space="PSUM") as psum_pool,
    ):
        # Resident operands (loaded once) on the scalar ring. hidT is split so
        # the first half (feeding the first head m-tiles) lands quickly.
        hid_sb = res.tile([128, KS, B_LOC], dt.bfloat16, tag="hid")
        nc.scalar.dma_start(hid_sb[:, :, :512], hid_t[:, :, :512])
        nc.scalar.dma_start(hid_sb[:, :, 512:], hid_t[:, :, 512:])
        p0_sb = res.tile([128, KS, H], dt.bfloat16, tag="p0")
        nc.scalar.dma_start(p0_sb[:], p0_t[:])
        d1w_sb = res.tile([128, KS, K1], dt.bfloat16, tag="d1w")
        nc.scalar.dma_start(d1w_sb[:], d1w_t[:])
        # Resident intermediates (tail decode matrices, K on partitions).
        dec0_sb = res.tile([128, KS, V0_LOC], dt.bfloat16, tag="dec0")
        dec1_sb = res.tile([128, KS1, V1_LOC], dt.bfloat16, tag="dec1")

        # G0: head = hidden_i @ Wcat.T  (kxm resident, kxn streamed on sync)
        off = 0
        for nw in wh:
            wt = kxn_pool.tile([128, KS, 512], dt.bfloat16, tag="kxn")
            nc.sync.dma_start(wt[:, :, :nw], wcat_t[:, :, ds(off, nw)])
            for m in range(B_LOC // 128):
                ps = psum_pool.tile([128, 512], dt.float32, tag="ps")
                for k in range(KS):
                    nc.tensor.matmul(
                        ps[:, :nw],
                        hid_sb[:, k, ts(m, 128)],
                        wt[:, k, :nw],
                        start=(k == 0),
                        stop=(k == KS - 1),
                    )
                st = stage_pool.tile([128, 512], dt.bfloat16, tag="st")
                nc.vector.tensor_copy(out=st[:, :nw], in_=ps[:, :nw])
                nc.sync.dma_start(head_o[ts(m, 128), ds(off, nw)], st[:, :nw])
            off += nw

        # Prefetch the first tail kxm tiles while the scalar ring is idle.
        h0_pre = {}
        p1_pre = {}
        for m in range(min(2, m0)):
            ht = kxm_pool.tile([128, KS, 128], dt.bfloat16, tag="h0")
            nc.scalar.dma_start(ht[:], h0_t[m])
            h0_pre[m] = ht
        for m in range(min(2, m1)):
            pt = kxm_pool.tile([128, KS1, 128], dt.bfloat16, tag="p1")
            nc.scalar.dma_start(pt[:], p1_t[m])
            p1_pre[m] = pt

        # G1: dec0'T = P0 @ embed0_i.T  -> resident dec0_sb
        off = 0
        for nw in w0:
            et = kxn_pool.tile([128, KS, 512], dt.bfloat16, tag="kxn")
            nc.sync.dma_start(et[:, :, :nw], e0_t[:, :, ds(off, nw)])
            for m in range(H // 128):
                ps = psum_pool.tile([128, 512], dt.float32, tag="ps")
                for k in range(KS):
                    nc.tensor.matmul(
                        ps[:, :nw],
                        p0_sb[:, k, ts(m, 128)],
                        et[:, k, :nw],
                        start=(k == 0),
                        stop=(k == KS - 1),
                    )
                nc.vector.tensor_copy(out=dec0_sb[:, m, ds(off, nw)], in_=ps[:, :nw])
            off += nw

        # G3: dec1T = down1_W @ embed1_i.T  -> resident dec1_sb
        off = 0
        for nw in w1:
            et = kxn_pool.tile([128, KS, 512], dt.bfloat16, tag="kxn")
            nc.sync.dma_start(et[:, :, :nw], e1_t[:, :, ds(off, nw)])
            for m in range(KS1):
                ps = psum_pool.tile([128, 512], dt.float32, tag="ps")
                for k in range(KS):
                    nc.tensor.matmul(
                        ps[:, :nw],
                        d1w_sb[:, k, ts(m, 128)],
                        et[:, k, :nw],
                        start=(k == 0),
                        stop=(k == KS - 1),
                    )
                nc.vector.tensor_copy(out=dec1_sb[:, m, ds(off, nw)], in_=ps[:, :nw])
            off += nw

        # G2/G4 m-tiles, interleaved so output-write DMA spreads evenly.
        # G2 psum eviction on DVE, G4's on ScalarE so neither engine saturates.
        def g2_mtile(m):
            ht = h0_pre.pop(m, None)
            if ht is None:
                ht = kxm_pool.tile([128, KS, 128], dt.bfloat16, tag="h0")
                nc.scalar.dma_start(ht[:], h0_t[m])
            row = stage_pool.tile([128, V0_LOC], dt.bfloat16, tag="st0")
            off = 0
            for nw in w0:
                ps = psum_pool.tile([128, 512], dt.float32, tag="ps")
                for k in range(KS):
                    nc.tensor.matmul(
                        ps[:, :nw],
                        ht[:, k, :],
                        dec0_sb[:, k, ds(off, nw)],
                        start=(k == 0),
                        stop=(k == KS - 1),
                    )
                nc.vector.tensor_copy(out=row[:, ds(off, nw)], in_=ps[:, :nw])
                off += nw
            nc.sync.dma_start(out0_o[ts(m, 128), :], row[:])

        def g4_mtile(m):
            pt = p1_pre.pop(m, None)
            if pt is None:
                pt = kxm_pool.tile([128, KS1, 128], dt.bfloat16, tag="p1")
                nc.scalar.dma_start(pt[:], p1_t[m])
            row = stage_pool.tile([128, V1_LOC], dt.bfloat16, tag="st1")
            off = 0
            for nw in w1:
                ps = psum_pool.tile([128, 512], dt.float32, tag="ps")
                for k in range(KS1):
                    nc.tensor.matmul(
                        ps[:, :nw],
                        pt[:, k, :],
                        dec1_sb[:, k, ds(off, nw)],
                        start=(k == 0),
                        stop=(k == KS1 - 1),
                    )
                nc.scalar.copy(out=row[:, ds(off, nw)], in_=ps[:, :nw])
                off += nw
            nc.sync.dma_start(out1_o[ts(m, 128), :], row[:])

        # Ratio-merge; G4 slightly ahead so the kernel tail ends on a small
        # G2 row store rather than a large G4 one.
        i0 = i1 = 0
        while i0 < m0 or i1 < m1:
            if i0 >= m0 or (i1 < m1 and i1 * m0 <= i0 * m1):
                g4_mtile(i1)
                i1 += 1
            else:
                g2_mtile(i0)
                i0 += 1

    nc.compile()
    return nc


def kernel(hidden, embed_weight, tail_vec_W, tail_vec_b, shortlist_bias,
           bias0, bias1, down0_W, down1_W, targets):
    global LAST_EXEC_NS
    _install_ntff_shim()
    from concourse.bass_utils import run_bass_kernel_spmd

    hidden = np.asarray(hidden, np.float32)
    embed_weight = np.asarray(embed_weight, np.float32)
    tail_vec_W = np.asarray(tail_vec_W, np.float32)
    tail_vec_b = np.asarray(tail_vec_b, np.float32)
    shortlist_bias = np.asarray(shortlist_bias, np.float32)
    bias0 = np.asarray(bias0, np.float32)
    bias1 = np.asarray(bias1, np.float32)
    down0_W = np.asarray(down0_W, np.float32)
    down1_W = np.asarray(down1_W, np.float32)
    t = np.asarray(targets)

    idx0 = np.nonzero((t >= C0) & (t < C1))[0]
    idx1 = np.nonzero((t >= C1) & (t < C2))[0]
    n0, n1 = len(idx0), len(idx1)
    n0p = max(128, -(-n0 // 128) * 128)
    n1p = max(128, -(-n1 // 128) * 128)

    # Host-side prep (cheap): gathers, concat, small GEMMs, bf16 packing.
    p0 = down0_W.T @ down0_W                                   # (H, H)
    hid0 = np.zeros((n0p, H), np.float32)
    hid0[:n0] = hidden[idx0]
    hid1 = hidden[idx1]
    proj1 = np.zeros((n1p, K1), np.float32)
    proj1[:n1] = hid1 @ down1_W.T
    wcat = np.concatenate([embed_weight[:C0], tail_vec_W], axis=0)  # (2002, H)

    wcat_t = _kx_tiled(np.ascontiguousarray(wcat.T))
    p0_t = _kx_tiled(p0)
    h0_t = _kx_mtiled(np.ascontiguousarray(hid0.T))
    d1w_t = _kx_tiled(np.ascontiguousarray(down1_W.T))
    p1_t = _kx_mtiled(np.ascontiguousarray(proj1.T))

    in_maps = []
    for i in range(N_CORES):
        hid_i = hidden[i * B_LOC:(i + 1) * B_LOC]
        e0_i = embed_weight[C0 + i * V0_LOC: C0 + (i + 1) * V0_LOC]
        e1_i = embed_weight[C1 + i * V1_LOC: C1 + (i + 1) * V1_LOC]
        in_maps.append({
            "hidT": _kx_tiled(np.ascontiguousarray(hid_i.T)),
            "wcatT": wcat_t,
            "p0T": p0_t,
            "e0T": _kx_tiled(np.ascontiguousarray(e0_i.T)),
            "h0T": h0_t,
            "d1WT": d1w_t,
            "e1T": _kx_tiled(np.ascontiguousarray(e1_i.T)),
            "p1T": p1_t,
        })

    key = (n0p, n1p)
    if key not in _GRAPH_CACHE:
        _GRAPH_CACHE[key] = _build_graph(n0p, n1p)
    nc = _GRAPH_CACHE[key]

    res = run_bass_kernel_spmd(
        nc, in_maps, core_ids=list(range(N_CORES)), trace=TRACE
    )
    LAST_EXEC_NS = res.exec_time_ns

    bh_full = np.concatenate([shortlist_bias, tail_vec_b]).astype(np.float32)
    head = np.concatenate(
        [np.asarray(res.results[i]["head_o"]).astype(np.float32)
         for i in range(N_CORES)], axis=0
    )
    head += bh_full[None, :]
    out0 = np.concatenate(
        [np.asarray(res.results[i]["out0_o"])[:n0].astype(np.float32)
         for i in range(N_CORES)], axis=1
    )
    out0 += bias0[None, :]
    out1 = np.concatenate(
        [np.asarray(res.results[i]["out1_o"])[:n1].astype(np.float32)
         for i in range(N_CORES)], axis=1
    )
    out1 += bias1[None, :]
    return (head, out0, out1)


# revision 13
# speedup vs baseline: 1.1809x; 1.1809x over previous
"""Adaptive-softmax logits (shortlist head + 2 tail clusters) on 8 TRN2 NeuronCores.

Sharding: head GEMM is data-parallel over tokens (1024 rows/core); both tail
clusters are vocab-sharded (each core computes 1/8 of each tail cluster's
columns for all tail tokens). No collectives: the host gathers tail rows,
packs per-core operands, and concatenates per-core outputs.

All device GEMMs run as out = lhsT.T @ rhs with bf16 operands and f32 PSUM
accumulation. Algebra: out0 = (h0 @ W0.T) @ (E0 @ W0.T).T = h0 @ (E0 @ P0).T
with P0 = W0.T @ W0 computed on host, so the per-token projection GEMM is not
replicated across cores. Biases are added on host during assembly; outputs
leave the device as bf16 to halve write bandwidth.

DMA engine split: weight/kxn streams and output stores ride the sync HWDGE
ring; resident and kxm loads ride the scalar HWDGE ring, so a stream with
stalled buffer slots can't head-of-line-block the other ring's loads.
"""

import sys
import types

import numpy as np
import ml_dtypes

BF16 = ml_dtypes.bfloat16

# Problem constants (hardcoded per task instructions).
N_CORES = 8
B, H, V = 8192, 1024, 50000
C0, C1, C2 = 2000, 20000, 50000   # cutoffs
V0 = C1 - C0                      # 18000 tail-0 vocab
V1 = C2 - C1                      # 30000 tail-1 vocab
V0_LOC = V0 // N_CORES            # 2250 per-core tail-0 columns
V1_LOC = V1 // N_CORES            # 3750
HEAD_N = 2002                     # 2000 shortlist + 2 tail logits
B_LOC = B // N_CORES              # 1024
KS = H // 128                     # 8 k-subtiles for K=1024
K1 = 256                          # contraction dim of tail-1 decode
KS1 = K1 // 128                   # 2

TRACE = False          # set True (e.g. from test.py) to neuron-profile the run
LAST_EXEC_NS = None    # filled when TRACE is on

_GRAPH_CACHE = {}


def _install_ntff_shim():
    """Register the NTFF profile hook bass_utils expects under axon.

    The agent image's ``antenv`` lacks ``axon_hooks``; recreate it in
    sys.modules backed by the ctypes driver from trn_agent_boot.
    """
    if "antenv.axon_hooks" in sys.modules:
        return
    m = types.ModuleType("antenv.axon_hooks")

    def set_axon_ntff_profile_hook(h):
        m._hook = h

    def get_axon_ntff_profile_hook():
        return getattr(m, "_hook", None)

    m.set_axon_ntff_profile_hook = set_axon_ntff_profile_hook
    m.get_axon_ntff_profile_hook = get_axon_ntff_profile_hook
    try:
        import antenv

        sys.modules["antenv.axon_hooks"] = m
        antenv.axon_hooks = m
        from trn_agent_boot.trn_boot import _ntff_profile_via_ctypes

        set_axon_ntff_profile_hook(
            _ntff_profile_via_ctypes("/opt/axon/libaxon_pjrt.so")
        )
    except Exception:
        pass


def _kx_tiled(a):
    """(K, M) -> (128, K//128, M) bf16: K on partitions, tiled by 128."""
    k, mdim = a.shape
    return np.ascontiguousarray(
        a.reshape(k // 128, 128, mdim).transpose(1, 0, 2)
    ).astype(BF16)


def _kx_mtiled(a):
    """(K, M) -> (M//128, 128, K//128, 128) bf16: one contiguous block per m-tile."""
    k, mdim = a.shape
    t = a.reshape(k // 128, 128, mdim // 128, 128)
    return np.ascontiguousarray(t.transpose(2, 1, 0, 3)).astype(BF16)


def _n_widths(total, tile):
    out = []
    off = 0
    while off < total:
        out.append(min(tile, total - off))
        off += tile
    return out


def _build_graph(n0p, n1p):
    import concourse.mybir as mybir
    import concourse.tile as tile
    from concourse import bacc
    from concourse.bass import ds, ts

    dt = mybir.dt
    m0 = n0p // 128
    m1 = n1p // 128

    nc = bacc.Bacc(None, target_bir_lowering=False, debug=False)

    hid_t = nc.declare_dram_parameter("hidT", [128, KS, B_LOC], dt.bfloat16, isOutput=False)
    wcat_t = nc.declare_dram_parameter("wcatT", [128, KS, HEAD_N], dt.bfloat16, isOutput=False)
    p0_t = nc.declare_dram_parameter("p0T", [128, KS, H], dt.bfloat16, isOutput=False)
    e0_t = nc.declare_dram_parameter("e0T", [128, KS, V0_LOC], dt.bfloat16, isOutput=False)
    h0_t = nc.declare_dram_parameter("h0T", [m0, 128, KS, 128], dt.bfloat16, isOutput=False)
    d1w_t = nc.declare_dram_parameter("d1WT", [128, KS, K1], dt.bfloat16, isOutput=False)
    e1_t = nc.declare_dram_parameter("e1T", [128, KS, V1_LOC], dt.bfloat16, isOutput=False)
    p1_t = nc.declare_dram_parameter("p1T", [m1, 128, KS1, 128], dt.bfloat16, isOutput=False)

    head_o = nc.declare_dram_parameter("head_o", [B_LOC, HEAD_N], dt.bfloat16, isOutput=True)
    out0_o = nc.declare_dram_parameter("out0_o", [n0p, V0_LOC], dt.bfloat16, isOutput=True)
    out1_o = nc.declare_dram_parameter("out1_o", [n1p, V1_LOC], dt.bfloat16, isOutput=True)

    wh = _n_widths(HEAD_N, 512)    # [512, 512, 512, 466]
    w0 = _n_widths(V0_LOC, 512)    # [512 x4, 202]
    w1 = _n_widths(V1_LOC, 512)    # [512 x7, 166]

    with (
        tile.TileContext(nc) as tc,
        tc.tile_pool(name="res", bufs=1) as res,
        tc.tile_pool(name="kxn", bufs=4) as kxn_pool,
        tc.tile_pool(name="kxm", bufs=6) as kxm_pool,
        tc.tile_pool(name="stage", bufs=3) as stage_pool,
        tc.tile_pool(name="psum", bufs=8, space="PSUM") as psum_pool,
    ):
        # Resident operands (loaded once) on the scalar ring. hidT is split so
        # the first half (feeding the first head m-tiles) lands quickly.
        hid_sb = res.tile([128, KS, B_LOC], dt.bfloat16, tag="hid")
        nc.scalar.dma_start(hid_sb[:, :, :256], hid_t[:, :, :256])
        nc.scalar.dma_start(hid_sb[:, :, 256:512], hid_t[:, :, 256:512])
        nc.scalar.dma_start(hid_sb[:, :, 512:], hid_t[:, :, 512:])
        p0_sb = res.tile([128, KS, H], dt.bfloat16, tag="p0")
        nc.scalar.dma_start(p0_sb[:], p0_t[:])
        d1w_sb = res.tile([128, KS, K1], dt.bfloat16, tag="d1w")
        nc.scalar.dma_start(d1w_sb[:], d1w_t[:])
        # Resident intermediates (tail decode matrices, K on partitions).
        dec0_sb = res.tile([128, KS, V0_LOC], dt.bfloat16, tag="dec0")
        dec1_sb = res.tile([128, KS1, V1_LOC], dt.bfloat16, tag="dec1")

        # G0: head = hidden_i @ Wcat.T  (kxm resident, kxn streamed on sync)
        off = 0
        for nw in wh:
            wt = kxn_pool.tile([128, KS, 512], dt.bfloat16, tag="kxn")
            nc.sync.dma_start(wt[:, :, :nw], wcat_t[:, :, ds(off, nw)])
            for m in range(B_LOC // 128):
                ps = psum_pool.tile([128, 512], dt.float32, tag="ps")
                for k in range(KS):
                    nc.tensor.matmul(
                        ps[:, :nw],
                        hid_sb[:, k, ts(m, 128)],
                        wt[:, k, :nw],
                        start=(k == 0),
                        stop=(k == KS - 1),
                    )
                st = stage_pool.tile([128, 512], dt.bfloat16, tag="st")
                nc.vector.tensor_copy(out=st[:, :nw], in_=ps[:, :nw])
                nc.sync.dma_start(head_o[ts(m, 128), ds(off, nw)], st[:, :nw])
            off += nw

        # Prefetch the first tail kxm tiles while the scalar ring is idle.
        h0_pre = {}
        p1_pre = {}
        for m in range(min(2, m0)):
            ht = kxm_pool.tile([128, KS, 128], dt.bfloat16, tag="h0")
            nc.scalar.dma_start(ht[:], h0_t[m])
            h0_pre[m] = ht
        for m in range(min(2, m1)):
            pt = kxm_pool.tile([128, KS1, 128], dt.bfloat16, tag="p1")
            nc.scalar.dma_start(pt[:], p1_t[m])
            p1_pre[m] = pt

        # G1: dec0'T = P0 @ embed0_i.T  -> resident dec0_sb
        off = 0
        for nw in w0:
            et = kxn_pool.tile([128, KS, 512], dt.bfloat16, tag="kxn")
            nc.sync.dma_start(et[:, :, :nw], e0_t[:, :, ds(off, nw)])
            for m in range(H // 128):
                ps = psum_pool.tile([128, 512], dt.float32, tag="ps")
                for k in range(KS):
                    nc.tensor.matmul(
                        ps[:, :nw],
                        p0_sb[:, k, ts(m, 128)],
                        et[:, k, :nw],
                        start=(k == 0),
                        stop=(k == KS - 1),
                    )
                nc.vector.tensor_copy(out=dec0_sb[:, m, ds(off, nw)], in_=ps[:, :nw])
            off += nw

        # G3: dec1T = down1_W @ embed1_i.T  -> resident dec1_sb
        off = 0
        for nw in w1:
            et = kxn_pool.tile([128, KS, 512], dt.bfloat16, tag="kxn")
            nc.sync.dma_start(et[:, :, :nw], e1_t[:, :, ds(off, nw)])
            for m in range(KS1):
                ps = psum_pool.tile([128, 512], dt.float32, tag="ps")
                for k in range(KS):
                    nc.tensor.matmul(
                        ps[:, :nw],
                        d1w_sb[:, k, ts(m, 128)],
                        et[:, k, :nw],
                        start=(k == 0),
                        stop=(k == KS - 1),
                    )
                nc.vector.tensor_copy(out=dec1_sb[:, m, ds(off, nw)], in_=ps[:, :nw])
            off += nw

        # G2/G4 m-tiles, interleaved so output-write DMA spreads evenly.
        # G2 psum eviction on DVE, G4's on ScalarE so neither engine saturates.
        def g2_mtile(m, final=False):
            ht = h0_pre.pop(m, None)
            if ht is None:
                ht = kxm_pool.tile([128, KS, 128], dt.bfloat16, tag="h0")
                nc.scalar.dma_start(ht[:], h0_t[m])
            row = stage_pool.tile([128, V0_LOC], dt.bfloat16, tag="st0")
            off = 0
            for nw in w0:
                ps = psum_pool.tile([128, 512], dt.float32, tag="ps")
                for k in range(KS):
                    nc.tensor.matmul(
                        ps[:, :nw],
                        ht[:, k, :],
                        dec0_sb[:, k, ds(off, nw)],
                        start=(k == 0),
                        stop=(k == KS - 1),
                    )
                nc.vector.tensor_copy(out=row[:, ds(off, nw)], in_=ps[:, :nw])
                if final:
                    # chunked store: the last small piece departs right after
                    # its eviction instead of waiting for the whole row
                    nc.sync.dma_start(out0_o[ts(m, 128), ds(off, nw)], row[:, ds(off, nw)])
                off += nw
            if not final:
                nc.sync.dma_start(out0_o[ts(m, 128), :], row[:])

        def g4_mtile(m):
            pt = p1_pre.pop(m, None)
            if pt is None:
                pt = kxm_pool.tile([128, KS1, 128], dt.bfloat16, tag="p1")
                nc.scalar.dma_start(pt[:], p1_t[m])
            row = stage_pool.tile([128, V1_LOC], dt.bfloat16, tag="st1")
            off = 0
            for nw in w1:
                ps = psum_pool.tile([128, 512], dt.float32, tag="ps")
                for k in range(KS1):
                    nc.tensor.matmul(
                        ps[:, :nw],
                        pt[:, k, :],
                        dec1_sb[:, k, ds(off, nw)],
                        start=(k == 0),
                        stop=(k == KS1 - 1),
                    )
                nc.scalar.copy(out=row[:, ds(off, nw)], in_=ps[:, :nw])
                off += nw
            nc.sync.dma_start(out1_o[ts(m, 128), :], row[:])

        # Ratio-merge; G4 slightly ahead so the kernel tail ends on a small
        # G2 row store rather than a large G4 one.
        i0 = i1 = 0
        while i0 < m0 or i1 < m1:
            if i0 >= m0 or (i1 < m1 and i1 * m0 <= i0 * m1):
                g4_mtile(i1)
                i1 += 1
            else:
                g2_mtile(i0, final=(i0 == m0 - 1 and i1 >= m1))
                i0 += 1

    nc.compile()
    return nc


def kernel(hidden, embed_weight, tail_vec_W, tail_vec_b, shortlist_bias,
           bias0, bias1, down0_W, down1_W, targets):
    global LAST_EXEC_NS
    _install_ntff_shim()
    from concourse.bass_utils import run_bass_kernel_spmd

    hidden = np.asarray(hidden, np.float32)
    embed_weight = np.asarray(embed_weight, np.float32)
    tail_vec_W = np.asarray(tail_vec_W, np.float32)
    tail_vec_b = np.asarray(tail_vec_b, np.float32)
    shortlist_bias = np.asarray(shortlist_bias, np.float32)
    bias0 = np.asarray(bias0, np.float32)
    bias1 = np.asarray(bias1, np.float32)
    down0_W = np.asarray(down0_W, np.float32)
    down1_W = np.asarray(down1_W, np.float32)
    t = np.asarray(targets)

    idx0 = np.nonzero((t >= C0) & (t < C1))[0]
    idx1 = np.nonzero((t >= C1) & (t < C2))[0]
    n0, n1 = len(idx0), len(idx1)
    n0p = max(128, -(-n0 // 128) * 128)
    n1p = max(128, -(-n1 // 128) * 128)

    # Host-side prep (cheap): gathers, concat, small GEMMs, bf16 packing.
    p0 = down0_W.T @ down0_W                                   # (H, H)
    hid0 = np.zeros((n0p, H), np.float32)
    hid0[:n0] = hidden[idx0]
    hid1 = hidden[idx1]
    proj1 = np.zeros((n1p, K1), np.float32)
    proj1[:n1] = hid1 @ down1_W.T
    wcat = np.concatenate([embed_weight[:C0], tail_vec_W], axis=0)  # (2002, H)

    wcat_t = _kx_tiled(np.ascontiguousarray(wcat.T))
    p0_t = _kx_tiled(p0)
    h0_t = _kx_mtiled(np.ascontiguousarray(hid0.T))
    d1w_t = _kx_tiled(np.ascontiguousarray(down1_W.T))
    p1_t = _kx_mtiled(np.ascontiguousarray(proj1.T))

    in_maps = []
    for i in range(N_CORES):
        hid_i = hidden[i * B_LOC:(i + 1) * B_LOC]
        e0_i = embed_weight[C0 + i * V0_LOC: C0 + (i + 1) * V0_LOC]
        e1_i = embed_weight[C1 + i * V1_LOC: C1 + (i + 1) * V1_LOC]
        in_maps.append({
            "hidT": _kx_tiled(np.ascontiguousarray(hid_i.T)),
            "wcatT": wcat_t,
            "p0T": p0_t,
            "e0T": _kx_tiled(np.ascontiguousarray(e0_i.T)),
            "h0T": h0_t,
            "d1WT": d1w_t,
            "e1T": _kx_tiled(np.ascontiguousarray(e1_i.T)),
            "p1T": p1_t,
        })

    key = (n0p, n1p)
    if key not in _GRAPH_CACHE:
        _GRAPH_CACHE[key] = _build_graph(n0p, n1p)
    nc = _GRAPH_CACHE[key]

    res = run_bass_kernel_spmd(
        nc, in_maps, core_ids=list(range(N_CORES)), trace=TRACE
    )
    LAST_EXEC_NS = res.exec_time_ns

    bh_full = np.concatenate([shortlist_bias, tail_vec_b]).astype(np.float32)
    head = np.concatenate(
        [np.asarray(res.results[i]["head_o"]).astype(np.float32)
         for i in range(N_CORES)], axis=0
    )
    head += bh_full[None, :]
    out0 = np.concatenate(
        [np.asarray(res.results[i]["out0_o"])[:n0].astype(np.float32)
         for i in range(N_CORES)], axis=1
    )
    out0 += bias0[None, :]
    out1 = np.concatenate(
        [np.asarray(res.results[i]["out1_o"])[:n1].astype(np.float32)
         for i in range(N_CORES)], axis=1
    )
    out1 += bias1[None, :]
    return (head, out0, out1)


# revision 14
# speedup vs baseline: 1.1861x; 1.0044x over previous
"""Adaptive-softmax logits (shortlist head + 2 tail clusters) on 8 TRN2 NeuronCores.

Sharding: head GEMM is data-parallel over tokens (1024 rows/core); both tail
clusters are vocab-sharded (each core computes 1/8 of each tail cluster's
columns for all tail tokens). No collectives: the host gathers tail rows,
packs per-core operands, and concatenates per-core outputs.

All device GEMMs run as out = lhsT.T @ rhs with bf16 operands and f32 PSUM
accumulation. Algebra: out0 = (h0 @ W0.T) @ (E0 @ W0.T).T = h0 @ (E0 @ P0).T
with P0 = W0.T @ W0 computed on host, so the per-token projection GEMM is not
replicated across cores. Biases are added on host during assembly; outputs
leave the device as bf16 to halve write bandwidth.

DMA engine split: weight/kxn streams and output stores ride the sync HWDGE
ring; resident and kxm loads ride the scalar HWDGE ring, so a stream with
stalled buffer slots can't head-of-line-block the other ring's loads.
"""

import sys
import types

import numpy as np
import ml_dtypes

BF16 = ml_dtypes.bfloat16

# Problem constants (hardcoded per task instructions).
N_CORES = 8
B, H, V = 8192, 1024, 50000
C0, C1, C2 = 2000, 20000, 50000   # cutoffs
V0 = C1 - C0                      # 18000 tail-0 vocab
V1 = C2 - C1                      # 30000 tail-1 vocab
V0_LOC = V0 // N_CORES            # 2250 per-core tail-0 columns
V1_LOC = V1 // N_CORES            # 3750
HEAD_N = 2002                     # 2000 shortlist + 2 tail logits
B_LOC = B // N_CORES              # 1024
KS = H // 128                     # 8 k-subtiles for K=1024
K1 = 256                          # contraction dim of tail-1 decode
KS1 = K1 // 128                   # 2

TRACE = False          # set True (e.g. from test.py) to neuron-profile the run
LAST_EXEC_NS = None    # filled when TRACE is on

_GRAPH_CACHE = {}


def _install_ntff_shim():
    """Register the NTFF profile hook bass_utils expects under axon.

    The agent image's ``antenv`` lacks ``axon_hooks``; recreate it in
    sys.modules backed by the ctypes driver from trn_agent_boot.
    """
    if "antenv.axon_hooks" in sys.modules:
        return
    m = types.ModuleType("antenv.axon_hooks")

    def set_axon_ntff_profile_hook(h):
        m._hook = h

    def get_axon_ntff_profile_hook():
        return getattr(m, "_hook", None)

    m.set_axon_ntff_profile_hook = set_axon_ntff_profile_hook
    m.get_axon_ntff_profile_hook = get_axon_ntff_profile_hook
    try:
        import antenv

        sys.modules["antenv.axon_hooks"] = m
        antenv.axon_hooks = m
        from trn_agent_boot.trn_boot import _ntff_profile_via_ctypes

        set_axon_ntff_profile_hook(
            _ntff_profile_via_ctypes("/opt/axon/libaxon_pjrt.so")
        )
    except Exception:
        pass


def _kx_tiled(a):
    """(K, M) -> (128, K//128, M) bf16: K on partitions, tiled by 128."""
    k, mdim = a.shape
    return np.ascontiguousarray(
        a.reshape(k // 128, 128, mdim).transpose(1, 0, 2)
    ).astype(BF16)


def _kx_mtiled(a):
    """(K, M) -> (M//128, 128, K//128, 128) bf16: one contiguous block per m-tile."""
    k, mdim = a.shape
    t = a.reshape(k // 128, 128, mdim // 128, 128)
    return np.ascontiguousarray(t.transpose(2, 1, 0, 3)).astype(BF16)


def _n_widths(total, tile):
    out = []
    off = 0
    while off < total:
        out.append(min(tile, total - off))
        off += tile
    return out


def _build_graph(n0p, n1p):
    import concourse.mybir as mybir
    import concourse.tile as tile
    from concourse import bacc
    from concourse.bass import ds, ts

    dt = mybir.dt
    m0 = n0p // 128
    m1 = n1p // 128

    nc = bacc.Bacc(None, target_bir_lowering=False, debug=False)

    hid_t = nc.declare_dram_parameter("hidT", [128, KS, B_LOC], dt.bfloat16, isOutput=False)
    wcat_t = nc.declare_dram_parameter("wcatT", [128, KS, HEAD_N], dt.bfloat16, isOutput=False)
    p0_t = nc.declare_dram_parameter("p0T", [128, KS, H], dt.bfloat16, isOutput=False)
    e0_t = nc.declare_dram_parameter("e0T", [128, KS, V0_LOC], dt.bfloat16, isOutput=False)
    h0_t = nc.declare_dram_parameter("h0T", [m0, 128, KS, 128], dt.bfloat16, isOutput=False)
    d1w_t = nc.declare_dram_parameter("d1WT", [128, KS, K1], dt.bfloat16, isOutput=False)
    e1_t = nc.declare_dram_parameter("e1T", [128, KS, V1_LOC], dt.bfloat16, isOutput=False)
    p1_t = nc.declare_dram_parameter("p1T", [m1, 128, KS1, 128], dt.bfloat16, isOutput=False)

    head_o = nc.declare_dram_parameter("head_o", [B_LOC, HEAD_N], dt.bfloat16, isOutput=True)
    out0_o = nc.declare_dram_parameter("out0_o", [n0p, V0_LOC], dt.bfloat16, isOutput=True)
    out1_o = nc.declare_dram_parameter("out1_o", [n1p, V1_LOC], dt.bfloat16, isOutput=True)

    wh = _n_widths(HEAD_N, 512)    # [512, 512, 512, 466]
    w0 = _n_widths(V0_LOC, 512)    # [512 x4, 202]
    w1 = _n_widths(V1_LOC, 512)    # [512 x7, 166]

    with (
        tile.TileContext(nc) as tc,
        tc.tile_pool(name="res", bufs=1) as res,
        tc.tile_pool(name="kxn", bufs=4) as kxn_pool,
        tc.tile_pool(name="kxm", bufs=6) as kxm_pool,
        tc.tile_pool(name="stage", bufs=3) as stage_pool,
        tc.tile_pool(name="psum", bufs=8, space="PSUM") as psum_pool,
    ):
        # Resident operands (loaded once) on the scalar ring. hidT is split so
        # the first half (feeding the first head m-tiles) lands quickly.
        hid_sb = res.tile([128, KS, B_LOC], dt.bfloat16, tag="hid")
        nc.scalar.dma_start(hid_sb[:, :, :512], hid_t[:, :, :512])
        nc.scalar.dma_start(hid_sb[:, :, 512:], hid_t[:, :, 512:])
        p0_sb = res.tile([128, KS, H], dt.bfloat16, tag="p0")
        nc.scalar.dma_start(p0_sb[:], p0_t[:])
        d1w_sb = res.tile([128, KS, K1], dt.bfloat16, tag="d1w")
        nc.scalar.dma_start(d1w_sb[:], d1w_t[:])
        # Resident intermediates (tail decode matrices, K on partitions).
        dec0_sb = res.tile([128, KS, V0_LOC], dt.bfloat16, tag="dec0")
        dec1_sb = res.tile([128, KS1, V1_LOC], dt.bfloat16, tag="dec1")

        # G0: head = hidden_i @ Wcat.T  (kxm resident, kxn streamed on sync)
        off = 0
        for nw in wh:
            wt = kxn_pool.tile([128, KS, 512], dt.bfloat16, tag="kxn")
            nc.sync.dma_start(wt[:, :, :nw], wcat_t[:, :, ds(off, nw)])
            for m in range(B_LOC // 128):
                ps = psum_pool.tile([128, 512], dt.float32, tag="ps")
                for k in range(KS):
                    nc.tensor.matmul(
                        ps[:, :nw],
                        hid_sb[:, k, ts(m, 128)],
                        wt[:, k, :nw],
                        start=(k == 0),
                        stop=(k == KS - 1),
                    )
                st = stage_pool.tile([128, 512], dt.bfloat16, tag="st")
                nc.vector.tensor_copy(out=st[:, :nw], in_=ps[:, :nw])
                nc.sync.dma_start(head_o[ts(m, 128), ds(off, nw)], st[:, :nw])
            off += nw

        # Prefetch the first tail kxm tiles while the scalar ring is idle.
        h0_pre = {}
        p1_pre = {}
        for m in range(min(2, m0)):
            ht = kxm_pool.tile([128, KS, 128], dt.bfloat16, tag="h0")
            nc.scalar.dma_start(ht[:], h0_t[m])
            h0_pre[m] = ht
        for m in range(min(2, m1)):
            pt = kxm_pool.tile([128, KS1, 128], dt.bfloat16, tag="p1")
            nc.scalar.dma_start(pt[:], p1_t[m])
            p1_pre[m] = pt

        # G1: dec0'T = P0 @ embed0_i.T  -> resident dec0_sb
        off = 0
        for nw in w0:
            et = kxn_pool.tile([128, KS, 512], dt.bfloat16, tag="kxn")
            nc.sync.dma_start(et[:, :, :nw], e0_t[:, :, ds(off, nw)])
            for m in range(H // 128):
                ps = psum_pool.tile([128, 512], dt.float32, tag="ps")
                for k in range(KS):
                    nc.tensor.matmul(
                        ps[:, :nw],
                        p0_sb[:, k, ts(m, 128)],
                        et[:, k, :nw],
                        start=(k == 0),
                        stop=(k == KS - 1),
                    )
                nc.vector.tensor_copy(out=dec0_sb[:, m, ds(off, nw)], in_=ps[:, :nw])
            off += nw

        # G3: dec1T = down1_W @ embed1_i.T  -> resident dec1_sb
        off = 0
        for nw in w1:
            et = kxn_pool.tile([128, KS, 512], dt.bfloat16, tag="kxn")
            nc.sync.dma_start(et[:, :, :nw], e1_t[:, :, ds(off, nw)])
            for m in range(KS1):
                ps = psum_pool.tile([128, 512], dt.float32, tag="ps")
                for k in range(KS):
                    nc.tensor.matmul(
                        ps[:, :nw],
                        d1w_sb[:, k, ts(m, 128)],
                        et[:, k, :nw],
                        start=(k == 0),
                        stop=(k == KS - 1),
                    )
                nc.vector.tensor_copy(out=dec1_sb[:, m, ds(off, nw)], in_=ps[:, :nw])
            off += nw

        # G2/G4 m-tiles, interleaved so output-write DMA spreads evenly.
        # G2 psum eviction on DVE, G4's on ScalarE so neither engine saturates.
        def g2_mtile(m):
            ht = h0_pre.pop(m, None)
            if ht is None:
                ht = kxm_pool.tile([128, KS, 128], dt.bfloat16, tag="h0")
                nc.scalar.dma_start(ht[:], h0_t[m])
            row = stage_pool.tile([128, V0_LOC], dt.bfloat16, tag="st0")
            off = 0
            for nw in w0:
                ps = psum_pool.tile([128, 512], dt.float32, tag="ps")
                for k in range(KS):
                    nc.tensor.matmul(
                        ps[:, :nw],
                        ht[:, k, :],
                        dec0_sb[:, k, ds(off, nw)],
                        start=(k == 0),
                        stop=(k == KS - 1),
                    )
                nc.vector.tensor_copy(out=row[:, ds(off, nw)], in_=ps[:, :nw])
                off += nw
            nc.sync.dma_start(out0_o[ts(m, 128), :], row[:])

        def g4_mtile(m):
            pt = p1_pre.pop(m, None)
            if pt is None:
                pt = kxm_pool.tile([128, KS1, 128], dt.bfloat16, tag="p1")
                nc.scalar.dma_start(pt[:], p1_t[m])
            row = stage_pool.tile([128, V1_LOC], dt.bfloat16, tag="st1")
            off = 0
            for nw in w1:
                ps = psum_pool.tile([128, 512], dt.float32, tag="ps")
                for k in range(KS1):
                    nc.tensor.matmul(
                        ps[:, :nw],
                        pt[:, k, :],
                        dec1_sb[:, k, ds(off, nw)],
                        start=(k == 0),
                        stop=(k == KS1 - 1),
                    )
                nc.scalar.copy(out=row[:, ds(off, nw)], in_=ps[:, :nw])
                off += nw
            nc.sync.dma_start(out1_o[ts(m, 128), :], row[:])

        # Ratio-merge; G4 slightly ahead so the kernel tail ends on a small
        # G2 row store rather than a large G4 one.
        i0 = i1 = 0
        while i0 < m0 or i1 < m1:
            if i0 >= m0 or (i1 < m1 and i1 * m0 <= i0 * m1):
                g4_mtile(i1)
                i1 += 1
            else:
                g2_mtile(i0)
                i0 += 1

    nc.compile()
    return nc


def kernel(hidden, embed_weight, tail_vec_W, tail_vec_b, shortlist_bias,
           bias0, bias1, down0_W, down1_W, targets):
    global LAST_EXEC_NS
    _install_ntff_shim()
    from concourse.bass_utils import run_bass_kernel_spmd

    hidden = np.asarray(hidden, np.float32)
    embed_weight = np.asarray(embed_weight, np.float32)
    tail_vec_W = np.asarray(tail_vec_W, np.float32)
    tail_vec_b = np.asarray(tail_vec_b, np.float32)
    shortlist_bias = np.asarray(shortlist_bias, np.float32)
    bias0 = np.asarray(bias0, np.float32)
    bias1 = np.asarray(bias1, np.float32)
    down0_W = np.asarray(down0_W, np.float32)
    down1_W = np.asarray(down1_W, np.float32)
    t = np.asarray(targets)

    idx0 = np.nonzero((t >= C0) & (t < C1))[0]
    idx1 = np.nonzero((t >= C1) & (t < C2))[0]
    n0, n1 = len(idx0), len(idx1)
    n0p = max(128, -(-n0 // 128) * 128)
    n1p = max(128, -(-n1 // 128) * 128)

    # Host-side prep (cheap): gathers, concat, small GEMMs, bf16 packing.
    p0 = down0_W.T @ down0_W                                   # (H, H)
    hid0 = np.zeros((n0p, H), np.float32)
    hid0[:n0] = hidden[idx0]
    hid1 = hidden[idx1]
    proj1 = np.zeros((n1p, K1), np.float32)
    proj1[:n1] = hid1 @ down1_W.T
    wcat = np.concatenate([embed_weight[:C0], tail_vec_W], axis=0)  # (2002, H)

    wcat_t = _kx_tiled(np.ascontiguousarray(wcat.T))
    p0_t = _kx_tiled(p0)
    h0_t = _kx_mtiled(np.ascontiguousarray(hid0.T))
    d1w_t = _kx_tiled(np.ascontiguousarray(down1_W.T))
    p1_t = _kx_mtiled(np.ascontiguousarray(proj1.T))

    in_maps = []
    for i in range(N_CORES):
        hid_i = hidden[i * B_LOC:(i + 1) * B_LOC]
        e0_i = embed_weight[C0 + i * V0_LOC: C0 + (i + 1) * V0_LOC]
        e1_i = embed_weight[C1 + i * V1_LOC: C1 + (i + 1) * V1_LOC]
        in_maps.append({
            "hidT": _kx_tiled(np.ascontiguousarray(hid_i.T)),
            "wcatT": wcat_t,
            "p0T": p0_t,
            "e0T": _kx_tiled(np.ascontiguousarray(e0_i.T)),
            "h0T": h0_t,
            "d1WT": d1w_t,
            "e1T": _kx_tiled(np.ascontiguousarray(e1_i.T)),
            "p1T": p1_t,
        })

    key = (n0p, n1p)
    if key not in _GRAPH_CACHE:
        _GRAPH_CACHE[key] = _build_graph(n0p, n1p)
    nc = _GRAPH_CACHE[key]

    res = run_bass_kernel_spmd(
        nc, in_maps, core_ids=list(range(N_CORES)), trace=TRACE
    )
    LAST_EXEC_NS = res.exec_time_ns

    bh_full = np.concatenate([shortlist_bias, tail_vec_b]).astype(np.float32)
    head = np.concatenate(
        [np.asarray(res.results[i]["head_o"]).astype(np.float32)
         for i in range(N_CORES)], axis=0
    )
    head += bh_full[None, :]
    out0 = np.concatenate(
        [np.asarray(res.results[i]["out0_o"])[:n0].astype(np.float32)
         for i in range(N_CORES)], axis=1
    )
    out0 += bias0[None, :]
    out1 = np.concatenate(
        [np.asarray(res.results[i]["out1_o"])[:n1].astype(np.float32)
         for i in range(N_CORES)], axis=1
    )
    out1 += bias1[None, :]
    return (head, out0, out1)


# revision 15
# speedup vs baseline: 1.1993x; 1.0111x over previous
"""Adaptive-softmax logits (shortlist head + 2 tail clusters) on 8 TRN2 NeuronCores.

Sharding: head GEMM is data-parallel over tokens (1024 rows/core); both tail
clusters are vocab-sharded (each core computes 1/8 of each tail cluster's
columns for all tail tokens). No collectives: the host gathers tail rows,
packs per-core operands, and concatenates per-core outputs.

All device GEMMs run as out = lhsT.T @ rhs with bf16 operands and f32 PSUM
accumulation. Algebra: out0 = (h0 @ W0.T) @ (E0 @ W0.T).T = h0 @ (E0 @ P0).T
with P0 = W0.T @ W0 computed on host, so the per-token projection GEMM is not
replicated across cores. Biases are added on host during assembly; outputs
leave the device as bf16 to halve write bandwidth.

DMA engine split: weight/kxn streams and output stores ride the sync HWDGE
ring; resident and kxm loads ride the scalar HWDGE ring, so a stream with
stalled buffer slots can't head-of-line-block the other ring's loads.
"""

import sys
import types

import numpy as np
import ml_dtypes

BF16 = ml_dtypes.bfloat16

# Problem constants (hardcoded per task instructions).
N_CORES = 8
B, H, V = 8192, 1024, 50000
C0, C1, C2 = 2000, 20000, 50000   # cutoffs
V0 = C1 - C0                      # 18000 tail-0 vocab
V1 = C2 - C1                      # 30000 tail-1 vocab
V0_LOC = V0 // N_CORES            # 2250 per-core tail-0 columns
V1_LOC = V1 // N_CORES            # 3750
HEAD_N = 2002                     # 2000 shortlist + 2 tail logits
B_LOC = B // N_CORES              # 1024
KS = H // 128                     # 8 k-subtiles for K=1024
K1 = 256                          # contraction dim of tail-1 decode
KS1 = K1 // 128                   # 2

TRACE = False          # set True (e.g. from test.py) to neuron-profile the run
LAST_EXEC_NS = None    # filled when TRACE is on

_GRAPH_CACHE = {}


def _install_ntff_shim():
    """Register the NTFF profile hook bass_utils expects under axon.

    The agent image's ``antenv`` lacks ``axon_hooks``; recreate it in
    sys.modules backed by the ctypes driver from trn_agent_boot.
    """
    if "antenv.axon_hooks" in sys.modules:
        return
    m = types.ModuleType("antenv.axon_hooks")

    def set_axon_ntff_profile_hook(h):
        m._hook = h

    def get_axon_ntff_profile_hook():
        return getattr(m, "_hook", None)

    m.set_axon_ntff_profile_hook = set_axon_ntff_profile_hook
    m.get_axon_ntff_profile_hook = get_axon_ntff_profile_hook
    try:
        import antenv

        sys.modules["antenv.axon_hooks"] = m
        antenv.axon_hooks = m
        from trn_agent_boot.trn_boot import _ntff_profile_via_ctypes

        set_axon_ntff_profile_hook(
            _ntff_profile_via_ctypes("/opt/axon/libaxon_pjrt.so")
        )
    except Exception:
        pass


def _kx_tiled(a):
    """(K, M) -> (128, K//128, M) bf16: K on partitions, tiled by 128."""
    k, mdim = a.shape
    return np.ascontiguousarray(
        a.reshape(k // 128, 128, mdim).transpose(1, 0, 2)
    ).astype(BF16)


def _kx_mtiled(a):
    """(K, M) -> (M//128, 128, K//128, 128) bf16: one contiguous block per m-tile."""
    k, mdim = a.shape
    t = a.reshape(k // 128, 128, mdim // 128, 128)
    return np.ascontiguousarray(t.transpose(2, 1, 0, 3)).astype(BF16)


def _n_widths(total, tile):
    out = []
    off = 0
    while off < total:
        out.append(min(tile, total - off))
        off += tile
    return out


def _build_graph(n0p, n1p):
    import concourse.mybir as mybir
    import concourse.tile as tile
    from concourse import bacc
    from concourse.bass import ds, ts

    dt = mybir.dt
    m0 = n0p // 128
    m1 = n1p // 128

    nc = bacc.Bacc(None, target_bir_lowering=False, debug=False)

    hid_t = nc.declare_dram_parameter("hidT", [128, KS, B_LOC], dt.bfloat16, isOutput=False)
    wcat_t = nc.declare_dram_parameter("wcatT", [128, KS, HEAD_N], dt.bfloat16, isOutput=False)
    p0_t = nc.declare_dram_parameter("p0T", [128, KS, H], dt.bfloat16, isOutput=False)
    e0_t = nc.declare_dram_parameter("e0T", [128, KS, V0_LOC], dt.bfloat16, isOutput=False)
    h0_t = nc.declare_dram_parameter("h0T", [m0, 128, KS, 128], dt.bfloat16, isOutput=False)
    d1w_t = nc.declare_dram_parameter("d1WT", [128, KS, K1], dt.bfloat16, isOutput=False)
    e1_t = nc.declare_dram_parameter("e1T", [128, KS, V1_LOC], dt.bfloat16, isOutput=False)
    p1_t = nc.declare_dram_parameter("p1T", [m1, 128, KS1, 128], dt.bfloat16, isOutput=False)

    head_o = nc.declare_dram_parameter("head_o", [B_LOC, HEAD_N], dt.bfloat16, isOutput=True)
    out0_o = nc.declare_dram_parameter("out0_o", [n0p, V0_LOC], dt.bfloat16, isOutput=True)
    out1_o = nc.declare_dram_parameter("out1_o", [n1p, V1_LOC], dt.bfloat16, isOutput=True)

    wh = _n_widths(HEAD_N, 512)    # [512, 512, 512, 466]
    w0 = _n_widths(V0_LOC, 512)    # [512 x4, 202]
    w1 = _n_widths(V1_LOC, 512)    # [512 x7, 166]

    with (
        tile.TileContext(nc) as tc,
        tc.tile_pool(name="res", bufs=1) as res,
        tc.tile_pool(name="kxn", bufs=4) as kxn_pool,
        tc.tile_pool(name="kxm", bufs=6) as kxm_pool,
        tc.tile_pool(name="stage", bufs=3) as stage_pool,
        tc.tile_pool(name="psum", bufs=8, space="PSUM") as psum_pool,
    ):
        # Resident operands (loaded once) on the scalar ring. hidT is split so
        # the first half (feeding the first head m-tiles) lands quickly.
        hid_sb = res.tile([128, KS, B_LOC], dt.bfloat16, tag="hid")
        nc.scalar.dma_start(hid_sb[:, :, :512], hid_t[:, :, :512])
        nc.scalar.dma_start(hid_sb[:, :, 512:], hid_t[:, :, 512:])
        p0_sb = res.tile([128, KS, H], dt.bfloat16, tag="p0")
        nc.scalar.dma_start(p0_sb[:], p0_t[:])
        d1w_sb = res.tile([128, KS, K1], dt.bfloat16, tag="d1w")
        nc.scalar.dma_start(d1w_sb[:], d1w_t[:])
        # Resident intermediates (tail decode matrices, K on partitions).
        dec0_sb = res.tile([128, KS, V0_LOC], dt.bfloat16, tag="dec0")
        dec1_sb = res.tile([128, KS1, V1_LOC], dt.bfloat16, tag="dec1")

        # G0: head = hidden_i @ Wcat.T  (kxm resident, kxn streamed on sync)
        off = 0
        for nw in wh:
            wt = kxn_pool.tile([128, KS, 512], dt.bfloat16, tag="kxn")
            nc.sync.dma_start(wt[:, :, :nw], wcat_t[:, :, ds(off, nw)])
            for m in range(B_LOC // 128):
                ps = psum_pool.tile([128, 512], dt.float32, tag="ps")
                for k in range(KS):
                    nc.tensor.matmul(
                        ps[:, :nw],
                        hid_sb[:, k, ts(m, 128)],
                        wt[:, k, :nw],
                        start=(k == 0),
                        stop=(k == KS - 1),
                    )
                st = stage_pool.tile([128, 512], dt.bfloat16, tag="st")
                nc.vector.tensor_copy(out=st[:, :nw], in_=ps[:, :nw])
                nc.sync.dma_start(head_o[ts(m, 128), ds(off, nw)], st[:, :nw])
            off += nw

        # Prefetch the first tail kxm tiles while the scalar ring is idle.
        h0_pre = {}
        p1_pre = {}
        for m in range(min(2, m0)):
            ht = kxm_pool.tile([128, KS, 128], dt.bfloat16, tag="h0")
            nc.scalar.dma_start(ht[:], h0_t[m])
            h0_pre[m] = ht
        for m in range(min(2, m1)):
            pt = kxm_pool.tile([128, KS1, 128], dt.bfloat16, tag="p1")
            nc.scalar.dma_start(pt[:], p1_t[m])
            p1_pre[m] = pt

        # G1: dec0'T = P0 @ embed0_i.T  -> resident dec0_sb
        off = 0
        for nw in w0:
            et = kxn_pool.tile([128, KS, 512], dt.bfloat16, tag="kxn")
            nc.sync.dma_start(et[:, :, :nw], e0_t[:, :, ds(off, nw)])
            for m in range(H // 128):
                ps = psum_pool.tile([128, 512], dt.float32, tag="ps")
                for k in range(KS):
                    nc.tensor.matmul(
                        ps[:, :nw],
                        p0_sb[:, k, ts(m, 128)],
                        et[:, k, :nw],
                        start=(k == 0),
                        stop=(k == KS - 1),
                    )
                nc.vector.tensor_copy(out=dec0_sb[:, m, ds(off, nw)], in_=ps[:, :nw])
            off += nw

        # G3: dec1T = down1_W @ embed1_i.T  -> resident dec1_sb
        off = 0
        for nw in w1:
            et = kxn_pool.tile([128, KS, 512], dt.bfloat16, tag="kxn")
            nc.sync.dma_start(et[:, :, :nw], e1_t[:, :, ds(off, nw)])
            for m in range(KS1):
                ps = psum_pool.tile([128, 512], dt.float32, tag="ps")
                for k in range(KS):
                    nc.tensor.matmul(
                        ps[:, :nw],
                        d1w_sb[:, k, ts(m, 128)],
                        et[:, k, :nw],
                        start=(k == 0),
                        stop=(k == KS - 1),
                    )
                nc.vector.tensor_copy(out=dec1_sb[:, m, ds(off, nw)], in_=ps[:, :nw])
            off += nw

        # G2/G4 m-tiles, interleaved so output-write DMA spreads evenly.
        # G2 psum eviction on DVE, G4's on ScalarE so neither engine saturates.
        def g2_mtile(m, final=False):
            ht = h0_pre.pop(m, None)
            if ht is None:
                ht = kxm_pool.tile([128, KS, 128], dt.bfloat16, tag="h0")
                nc.scalar.dma_start(ht[:], h0_t[m])
            row = stage_pool.tile([128, V0_LOC], dt.bfloat16, tag="st0")
            off = 0
            for nw in w0:
                ps = psum_pool.tile([128, 512], dt.float32, tag="ps")
                for k in range(KS):
                    nc.tensor.matmul(
                        ps[:, :nw],
                        ht[:, k, :],
                        dec0_sb[:, k, ds(off, nw)],
                        start=(k == 0),
                        stop=(k == KS - 1),
                    )
                nc.vector.tensor_copy(out=row[:, ds(off, nw)], in_=ps[:, :nw])
                if final:
                    # chunked store: each piece departs right after its
                    # eviction so the kernel tail isn't gated on a full row
                    nc.sync.dma_start(out0_o[ts(m, 128), ds(off, nw)], row[:, ds(off, nw)])
                off += nw
            if not final:
                nc.sync.dma_start(out0_o[ts(m, 128), :], row[:])

        def g4_mtile(m):
            pt = p1_pre.pop(m, None)
            if pt is None:
                pt = kxm_pool.tile([128, KS1, 128], dt.bfloat16, tag="p1")
                nc.scalar.dma_start(pt[:], p1_t[m])
            row = stage_pool.tile([128, V1_LOC], dt.bfloat16, tag="st1")
            off = 0
            for nw in w1:
                ps = psum_pool.tile([128, 512], dt.float32, tag="ps")
                for k in range(KS1):
                    nc.tensor.matmul(
                        ps[:, :nw],
                        pt[:, k, :],
                        dec1_sb[:, k, ds(off, nw)],
                        start=(k == 0),
                        stop=(k == KS1 - 1),
                    )
                nc.scalar.copy(out=row[:, ds(off, nw)], in_=ps[:, :nw])
                off += nw
            nc.sync.dma_start(out1_o[ts(m, 128), :], row[:])

        # Ratio-merge; G4 slightly ahead so the kernel tail ends on a small
        # G2 row store rather than a large G4 one.
        i0 = i1 = 0
        while i0 < m0 - 1 or i1 < m1:
            if i0 >= m0 - 1 or (i1 < m1 and i1 * m0 <= i0 * m1):
                g4_mtile(i1)
                i1 += 1
            else:
                g2_mtile(i0)
                i0 += 1
        g2_mtile(m0 - 1, final=True)

    nc.compile()
    return nc


def kernel(hidden, embed_weight, tail_vec_W, tail_vec_b, shortlist_bias,
           bias0, bias1, down0_W, down1_W, targets):
    global LAST_EXEC_NS
    _install_ntff_shim()
    from concourse.bass_utils import run_bass_kernel_spmd

    hidden = np.asarray(hidden, np.float32)
    embed_weight = np.asarray(embed_weight, np.float32)
    tail_vec_W = np.asarray(tail_vec_W, np.float32)
    tail_vec_b = np.asarray(tail_vec_b, np.float32)
    shortlist_bias = np.asarray(shortlist_bias, np.float32)
    bias0 = np.asarray(bias0, np.float32)
    bias1 = np.asarray(bias1, np.float32)
    down0_W = np.asarray(down0_W, np.float32)
    down1_W = np.asarray(down1_W, np.float32)
    t = np.asarray(targets)

    idx0 = np.nonzero((t >= C0) & (t < C1))[0]
    idx1 = np.nonzero((t >= C1) & (t < C2))[0]
    n0, n1 = len(idx0), len(idx1)
    n0p = max(128, -(-n0 // 128) * 128)
    n1p = max(128, -(-n1 // 128) * 128)

    # Host-side prep (cheap): gathers, concat, small GEMMs, bf16 packing.
    p0 = down0_W.T @ down0_W                                   # (H, H)
    hid0 = np.zeros((n0p, H), np.float32)
    hid0[:n0] = hidden[idx0]
    hid1 = hidden[idx1]
    proj1 = np.zeros((n1p, K1), np.float32)
    proj1[:n1] = hid1 @ down1_W.T
    wcat = np.concatenate([embed_weight[:C0], tail_vec_W], axis=0)  # (2002, H)

    wcat_t = _kx_tiled(np.ascontiguousarray(wcat.T))
    p0_t = _kx_tiled(p0)
    h0_t = _kx_mtiled(np.ascontiguousarray(hid0.T))
    d1w_t = _kx_tiled(np.ascontiguousarray(down1_W.T))
    p1_t = _kx_mtiled(np.ascontiguousarray(proj1.T))

    in_maps = []
    for i in range(N_CORES):
        hid_i = hidden[i * B_LOC:(i + 1) * B_LOC]
        e0_i = embed_weight[C0 + i * V0_LOC: C0 + (i + 1) * V0_LOC]
        e1_i = embed_weight[C1 + i * V1_LOC: C1 + (i + 1) * V1_LOC]
        in_maps.append({
            "hidT": _kx_tiled(np.ascontiguousarray(hid_i.T)),
            "wcatT": wcat_t,
            "p0T": p0_t,
            "e0T": _kx_tiled(np.ascontiguousarray(e0_i.T)),
            "h0T": h0_t,
            "d1WT": d1w_t,
            "e1T": _kx_tiled(np.ascontiguousarray(e1_i.T)),
            "p1T": p1_t,
        })

    key = (n0p, n1p)
    if key not in _GRAPH_CACHE:
        _GRAPH_CACHE[key] = _build_graph(n0p, n1p)
    nc = _GRAPH_CACHE[key]

    res = run_bass_kernel_spmd(
        nc, in_maps, core_ids=list(range(N_CORES)), trace=TRACE
    )
    LAST_EXEC_NS = res.exec_time_ns

    bh_full = np.concatenate([shortlist_bias, tail_vec_b]).astype(np.float32)
    head = np.concatenate(
        [np.asarray(res.results[i]["head_o"]).astype(np.float32)
         for i in range(N_CORES)], axis=0
    )
    head += bh_full[None, :]
    out0 = np.concatenate(
        [np.asarray(res.results[i]["out0_o"])[:n0].astype(np.float32)
         for i in range(N_CORES)], axis=1
    )
    out0 += bias0[None, :]
    out1 = np.concatenate(
        [np.asarray(res.results[i]["out1_o"])[:n1].astype(np.float32)
         for i in range(N_CORES)], axis=1
    )
    out1 += bias1[None, :]
    return (head, out0, out1)
